# revision 20
# baseline (speedup 1.0000x reference)
"""Trainium2 Bass kernel for nn_ExpertFFN (top-1 MoE, B=4 S=2048 H=1024 E=8).

Strategy: EXPERT parallelism.  Core c owns expert c's weights only (bf16,
2 MB instead of 32 MB replicated), and the host does all routing:

  host:   logits = x @ router_w + router_b (fp32, same as the reference),
          idx = argmax, gate = softmax max = 1/sum(exp(l - max)).
          Tokens for expert e are gathered, pre-scaled by gate (y =
          gate*(x@W+b) = (gate*x)@W + gate*b), transposed to feature-major,
          cast to bf16, zero-padded to a shared capacity, and laid out
          k-chunk-major ([128, KC*n]) so the device loads each stream with
          4 large contiguous DMAs (sizes 1/1/2/4 chunks: fine-grained at
          the front for early compute start, big at the back for
          bandwidth; 8 HWDGE DMAs in flight total avoids sem-lane reuse
          stalls).
  device: pure single-expert GEMM  y[cap, H] = xT.T @ W  in bf16 with fp32
          PSUM accumulation.  Tokens ride the stationary operand (lhsT =
          xT column tile), weight columns stream as rhs (2 x 512-col PSUM
          banks per token tile).  The first 4 token tiles run (k-chunk)-
          major so each arriving chunk feeds ~2us of PE work; remaining
          tiles run t-major so evacuation (vector/scalar halves, y halves
          DMA'd on the gpsimd + sync queues) hides behind the next tile's
          matmuls and the kernel tail is one half-tile evacuation.
  host:   scatter y rows back by token index, add gate*expert_b, unshard.

Pure-bf16 precision measured at rel err 2.3e-3 vs the fp32 reference
(tolerance 2e-2).
"""

import sys

for _p in ("/opt/trn_rl_repo",):
    if _p not in sys.path:
        sys.path.insert(0, _p)

import numpy as np

P = 128
H = 1024
E = 8
NCORES = 8
KC = H // P          # contraction chunks
GRP = 4              # token tiles in the k-major (DMA-overlap) group
SPLITS = (1, 1, 1, 1, 4)  # chunks per input DMA (first = combined w0+x0)


def _build(cap: int):
    import concourse.mybir as mybir
    import concourse.tile as tile
    from concourse import bacc

    f32 = mybir.dt.float32
    bf16 = mybir.dt.bfloat16

    ntt = (cap + P - 1) // P

    nc = bacc.Bacc("TRN2", target_bir_lowering=False, debug=False,
                   num_devices=NCORES)

    # host pre-arranged chunk-major layouts: [p, k*N + j] = src[k*128+p, j].
    # Chunk 0 of both streams is packed into one tensor so the critical
    # first transfer is a single DMA on one ring.
    wx0_d = nc.dram_tensor("wx0", [P, H + cap], bf16, kind="ExternalInput")
    w_d = nc.dram_tensor("w", [P, (KC - 1) * H], bf16, kind="ExternalInput")
    xt_d = nc.dram_tensor("xt", [P, (KC - 1) * cap], bf16,
                          kind="ExternalInput")
    y_d = nc.dram_tensor("y", [cap, H], f32, kind="ExternalOutput")

    with tile.TileContext(nc) as tc:
        with (
            tc.tile_pool(name="consts", bufs=1) as cpool,
            tc.tile_pool(name="wp", bufs=len(SPLITS)) as wpool,
            tc.tile_pool(name="xp", bufs=len(SPLITS)) as xpool,
            tc.tile_pool(name="yp", bufs=ntt + 1) as ypool,
            tc.tile_pool(name="ps", bufs=GRP, space="PSUM") as pspool,
        ):
            # HAM warmup: dummy matmul activity from t=0 so the PE clock
            # gate opens while the input DMAs land; memset on vector (its
            # IRAM loads early) so the warmup starts ~5us, sized to span
            # until the first weight chunk arrives (~11.5us)
            warm = cpool.tile([P, 128], bf16)
            nc.vector.memset(warm[:], 0.0)
            pw = pspool.tile([P, 128], f32, tag="ps", space="PSUM")
            NWARM = 36
            for i in range(NWARM):
                nc.tensor.matmul(out=pw[:], lhsT=warm[:], rhs=warm[:],
                                 start=(i == 0), stop=(i == NWARM - 1))

            # graduated input DMAs on the two HWDGE rings: the combined
            # (w0, x0) block first on sync, then chunks 1..7 in growing
            # splits (1, 2, 4) per stream
            wx0 = wpool.tile([P, H + cap], bf16, tag="wx0", name="wx0")
            nc.sync.dma_start(out=wx0[:], in_=wx0_d[:, :])
            wq, xq, qof = [wx0], [wx0], [0]
            off = 0
            for s in SPLITS[1:]:
                wt = wpool.tile([P, s * H], bf16, tag=f"w{off}",
                                name=f"w{off}")
                nc.sync.dma_start(out=wt[:],
                                  in_=w_d[:, off * H:(off + s) * H])
                xt = xpool.tile([P, s * cap], bf16, tag=f"x{off}",
                                name=f"x{off}")
                nc.scalar.dma_start(out=xt[:],
                                    in_=xt_d[:, off * cap:(off + s) * cap])
                for _ in range(s):
                    wq.append(wt)
                    xq.append(xt)
                    qof.append(off)
                off += s

            def wk(k):
                if k == 0:
                    return wx0[:, 0:H]
                return wq[k][:, (k - 1 - qof[k]) * H:(k - qof[k]) * H]

            def xk(k):
                if k == 0:
                    return wx0[:, H:H + cap]
                return xq[k][:, (k - 1 - qof[k]) * cap:(k - qof[k]) * cap]

            ps = {}

            def evac(t, last=False):
                tw = min(P, cap - t * P)
                yb = ypool.tile([P, H], f32, tag="y", name=f"y{t}")
                nc.vector.tensor_copy(out=yb[0:tw, 0:512],
                                      in_=ps[t][0:tw, 0:512])
                nc.scalar.copy(out=yb[0:tw, 512:H], in_=ps[t][0:tw, 512:H])
                if last:
                    # half-column writes on the two low-latency HWDGE
                    # queues (one DMA issue each) so the tail is short
                    nc.sync.dma_start(out=y_d[t * P:t * P + tw, 0:512],
                                      in_=yb[0:tw, 0:512])
                    nc.scalar.dma_start(out=y_d[t * P:t * P + tw, 512:H],
                                        in_=yb[0:tw, 512:H])
                else:
                    nc.gpsimd.dma_start(out=y_d[t * P:t * P + tw, 0:512],
                                        in_=yb[0:tw, 0:512])
                    nc.sync.dma_start(out=y_d[t * P:t * P + tw, 512:H],
                                      in_=yb[0:tw, 512:H])

            # group A: k-major over the first GRP token tiles so each
            # arriving chunk feeds 8 matmuls
            ga = range(0, min(GRP, ntt))
            for t in ga:
                ps[t] = pspool.tile([P, H], f32, tag="ps", space="PSUM",
                                    name=f"ps{t}")
            for k in range(KC):
                for t in ga:
                    tw = min(P, cap - t * P)
                    for n in range(2):
                        nc.tensor.matmul(
                            out=ps[t][0:tw, n * 512:(n + 1) * 512],
                            lhsT=xk(k)[:, t * P:t * P + tw],
                            rhs=wk(k)[:, n * 512:(n + 1) * 512],
                            start=(k == 0), stop=(k == KC - 1))
            for t in ga:
                evac(t)

            # remaining tiles: t-major; evacuation hides behind the next
            # tile's matmuls
            for t in range(GRP, ntt):
                tw = min(P, cap - t * P)
                ps[t] = pspool.tile([P, H], f32, tag="ps", space="PSUM",
                                    name=f"ps{t}")
                for k in range(KC):
                    for n in range(2):
                        nc.tensor.matmul(
                            out=ps[t][0:tw, n * 512:(n + 1) * 512],
                            lhsT=xk(k)[:, t * P:t * P + tw],
                            rhs=wk(k)[:, n * 512:(n + 1) * 512],
                            start=(k == 0), stop=(k == KC - 1))
                evac(t, last=(t == ntt - 1))

    nc.compile()
    return nc


_NC_CACHE = {}


def _get_nc(cap: int):
    if cap not in _NC_CACHE:
        _NC_CACHE[cap] = _build(cap)
    return _NC_CACHE[cap]


def plan(x, router_w, router_b):
    """Host-side routing: token lists per expert, gate values, capacity."""
    xt = x.reshape(-1, H)
    logits = xt @ router_w + router_b
    idx = logits.argmax(-1)
    mx = logits.max(-1)
    gate = 1.0 / np.exp(logits - mx[:, None]).sum(-1)
    toks = [np.where(idx == e)[0] for e in range(E)]
    cap = max(P, -(-max(len(t) for t in toks) // 64) * 64)
    return toks, gate.astype(np.float32), cap


def make_in_maps(x, expert_w, toks, gate, cap):
    import ml_dtypes
    bf = ml_dtypes.bfloat16
    xt = x.reshape(-1, H)
    maps = []
    for e in range(E):
        te = toks[e]
        xs = np.zeros((KC, P, cap), dtype=bf)
        xs.reshape(H, cap)[:, :len(te)] = (xt[te] * gate[te, None]).T.astype(bf)
        w = expert_w[e].astype(bf).reshape(KC, P, H)
        w_arr = w.transpose(1, 0, 2).reshape(P, KC * H)
        x_arr = xs.transpose(1, 0, 2).reshape(P, KC * cap)
        maps.append({
            "wx0": np.ascontiguousarray(
                np.concatenate([w_arr[:, :H], x_arr[:, :cap]], axis=1)),
            "w": np.ascontiguousarray(w_arr[:, H:]),
            "xt": np.ascontiguousarray(x_arr[:, cap:]),
        })
    return maps


def assemble(results, toks, gate, expert_b, shape):
    T = shape[0] * shape[1]
    y = np.empty((T, H), dtype=np.float32)
    for e in range(E):
        te = toks[e]
        y[te] = results[e]["y"][:len(te)]
        if expert_b is not None:
            y[te] += gate[te, None] * expert_b[e][None, :]
    return y.reshape(shape)


def kernel(x, router_w, router_b, expert_w, expert_b):
    from concourse.bass_utils import run_bass_kernel_spmd

    x = np.ascontiguousarray(np.asarray(x, dtype=np.float32))
    router_w = np.ascontiguousarray(np.asarray(router_w, dtype=np.float32))
    router_b = np.ascontiguousarray(np.asarray(router_b, dtype=np.float32))
    expert_w = np.ascontiguousarray(np.asarray(expert_w, dtype=np.float32))
    expert_b = np.ascontiguousarray(np.asarray(expert_b, dtype=np.float32))

    B, S, Hx = x.shape
    assert Hx == H and B * S % NCORES == 0, (x.shape,)

    toks, gate, cap = plan(x, router_w, router_b)
    nc = _get_nc(cap)
    in_maps = make_in_maps(x, expert_w, toks, gate, cap)
    res = run_bass_kernel_spmd(nc, in_maps, list(range(NCORES)))
    eb = expert_b if np.any(expert_b != 0) else None
    return assemble(res.results, toks, gate, eb, (B, S, H))


# revision 25
# speedup vs baseline: 1.1501x; 1.1501x over previous
"""Trainium2 Bass kernel for nn_ExpertFFN (top-1 MoE, B=4 S=2048 H=1024 E=8).

Strategy: EXPERT parallelism.  Core c owns expert c's weights only (bf16,
2 MB instead of 32 MB replicated), and the host does all routing:

  host:   logits = x @ router_w + router_b (fp32, same as the reference),
          idx = argmax, gate = softmax max = 1/sum(exp(l - max)).
          Tokens for expert e are gathered, pre-scaled by gate (y =
          gate*(x@W+b) = (gate*x)@W + gate*b), transposed to feature-major,
          cast to bf16, zero-padded to a shared capacity, and laid out
          k-chunk-major ([128, KC*n]) so the device loads each stream with
          4 large contiguous DMAs (sizes 1/1/2/4 chunks: fine-grained at
          the front for early compute start, big at the back for
          bandwidth; 8 HWDGE DMAs in flight total avoids sem-lane reuse
          stalls).
  device: pure single-expert GEMM  y[cap, H] = xT.T @ W  in bf16 with fp32
          PSUM accumulation.  Tokens ride the stationary operand (lhsT =
          xT column tile), weight columns stream as rhs (2 x 512-col PSUM
          banks per token tile).  The first 4 token tiles run (k-chunk)-
          major so each arriving chunk feeds ~2us of PE work; remaining
          tiles run t-major so evacuation (vector/scalar halves, y halves
          DMA'd on the gpsimd + sync queues) hides behind the next tile's
          matmuls and the kernel tail is one half-tile evacuation.
  host:   scatter y rows back by token index, add gate*expert_b, unshard.

Pure-bf16 precision measured at rel err 2.3e-3 vs the fp32 reference
(tolerance 2e-2).
"""

import sys

for _p in ("/opt/trn_rl_repo",):
    if _p not in sys.path:
        sys.path.insert(0, _p)

import numpy as np

P = 128
H = 1024
E = 8
NCORES = 8
KC = H // P          # contraction chunks
GRP = 4              # token tiles in the k-major (DMA-overlap) group
SPLITS = (1, 1, 1, 1, 4)  # chunks per input DMA (first = combined w0+x0)


def _build(cap: int):
    import concourse.mybir as mybir
    import concourse.tile as tile
    from concourse import bacc

    f32 = mybir.dt.float32
    bf16 = mybir.dt.bfloat16

    ntt = (cap + P - 1) // P

    nc = bacc.Bacc("TRN2", target_bir_lowering=False, debug=False,
                   num_devices=NCORES)

    # host pre-arranged chunk-major layouts: [p, k*N + j] = src[k*128+p, j].
    # The critical first transfer is one DMA carrying w chunk 0 plus the
    # first 512 token columns of x chunk 0 (all that the k-major group
    # needs); x0's remaining columns lead the second stream.
    X0B = cap - 512
    wx0_d = nc.dram_tensor("wx0", [P, H + 512], bf16, kind="ExternalInput")
    w_d = nc.dram_tensor("w", [P, (KC - 1) * H], bf16, kind="ExternalInput")
    xt_d = nc.dram_tensor("xt", [P, X0B + (KC - 1) * cap], bf16,
                          kind="ExternalInput")
    y_d = nc.dram_tensor("y", [cap, H], f32, kind="ExternalOutput")

    with tile.TileContext(nc) as tc:
        with (
            tc.tile_pool(name="consts", bufs=1) as cpool,
            tc.tile_pool(name="wp", bufs=len(SPLITS)) as wpool,
            tc.tile_pool(name="xp", bufs=len(SPLITS)) as xpool,
            tc.tile_pool(name="yp", bufs=ntt + 1) as ypool,
            tc.tile_pool(name="ps", bufs=GRP, space="PSUM") as pspool,
        ):
            # HAM warmup: dummy matmul activity from t=0 so the PE clock
            # gate opens while the input DMAs land; memset on vector (its
            # IRAM loads early) so the warmup starts ~5us, sized to span
            # until the first weight chunk arrives (~11.5us)
            warm = cpool.tile([P, 128], bf16)
            nc.vector.memset(warm[:], 0.0)
            pw = pspool.tile([P, 128], f32, tag="ps", space="PSUM")
            NWARM = 36
            for i in range(NWARM):
                nc.tensor.matmul(out=pw[:], lhsT=warm[:], rhs=warm[:],
                                 start=(i == 0), stop=(i == NWARM - 1))

            # graduated input DMAs on the two HWDGE rings: the combined
            # (w0, x0-front) block first on sync, then chunks 1..7 in
            # splits (1, 1, 1, 4) per stream; the first scalar DMA also
            # carries x0's remaining token columns
            wx0 = wpool.tile([P, H + 512], bf16, tag="wx0", name="wx0")
            nc.sync.dma_start(out=wx0[:], in_=wx0_d[:, :])
            wq, xq, qof = [wx0], [None], [0]
            off = 0
            for si, s in enumerate(SPLITS[1:]):
                wt = wpool.tile([P, s * H], bf16, tag=f"w{off}",
                                name=f"w{off}")
                nc.sync.dma_start(out=wt[:],
                                  in_=w_d[:, off * H:(off + s) * H])
                xb = X0B if si == 0 else 0
                xt = xpool.tile([P, xb + s * cap], bf16, tag=f"x{off}",
                                name=f"x{off}")
                nc.scalar.dma_start(
                    out=xt[:],
                    in_=xt_d[:, off * cap + (X0B - xb):
                             X0B + (off + s) * cap])
                if si == 0:
                    x0b = xt
                for _ in range(s):
                    wq.append(wt)
                    xq.append(xt)
                    qof.append(off)
                off += s

            def wk(k):
                if k == 0:
                    return wx0[:, 0:H]
                return wq[k][:, (k - 1 - qof[k]) * H:(k - qof[k]) * H]

            def xk(k, tcols):
                # chunk k's token columns [tcols.start, tcols.stop)
                if k == 0:
                    if tcols.stop <= 512:
                        return wx0[:, H + tcols.start:H + tcols.stop]
                    return x0b[:, tcols.start - 512:tcols.stop - 512]
                base = (X0B if qof[k] == 0 else 0) + (k - 1 - qof[k]) * cap
                return xq[k][:, base + tcols.start:base + tcols.stop]

            ps = {}

            def evac(t, last=False):
                tw = min(P, cap - t * P)
                yb = ypool.tile([P, H], f32, tag="y", name=f"y{t}")
                nc.vector.tensor_copy(out=yb[0:tw, 0:512],
                                      in_=ps[t][0:tw, 0:512])
                nc.scalar.copy(out=yb[0:tw, 512:H], in_=ps[t][0:tw, 512:H])
                if last:
                    # half-column writes on the two low-latency HWDGE
                    # queues (one DMA issue each) so the tail is short
                    nc.sync.dma_start(out=y_d[t * P:t * P + tw, 0:512],
                                      in_=yb[0:tw, 0:512])
                    nc.scalar.dma_start(out=y_d[t * P:t * P + tw, 512:H],
                                        in_=yb[0:tw, 512:H])
                else:
                    nc.gpsimd.dma_start(out=y_d[t * P:t * P + tw, 0:512],
                                        in_=yb[0:tw, 0:512])
                    nc.sync.dma_start(out=y_d[t * P:t * P + tw, 512:H],
                                      in_=yb[0:tw, 512:H])

            # group A: k-major over the first GRP token tiles so each
            # arriving chunk feeds 8 matmuls
            ga = range(0, min(GRP, ntt))
            for t in ga:
                ps[t] = pspool.tile([P, H], f32, tag="ps", space="PSUM",
                                    name=f"ps{t}")
            for k in range(KC):
                for t in ga:
                    tw = min(P, cap - t * P)
                    for n in range(2):
                        nc.tensor.matmul(
                            out=ps[t][0:tw, n * 512:(n + 1) * 512],
                            lhsT=xk(k, slice(t * P, t * P + tw)),
                            rhs=wk(k)[:, n * 512:(n + 1) * 512],
                            start=(k == 0), stop=(k == KC - 1))
            for t in ga:
                evac(t)

            # remaining tiles: t-major; evacuation hides behind the next
            # tile's matmuls
            for t in range(GRP, ntt):
                tw = min(P, cap - t * P)
                ps[t] = pspool.tile([P, H], f32, tag="ps", space="PSUM",
                                    name=f"ps{t}")
                for k in range(KC):
                    for n in range(2):
                        nc.tensor.matmul(
                            out=ps[t][0:tw, n * 512:(n + 1) * 512],
                            lhsT=xk(k, slice(t * P, t * P + tw)),
                            rhs=wk(k)[:, n * 512:(n + 1) * 512],
                            start=(k == 0), stop=(k == KC - 1))
                evac(t, last=(t == ntt - 1))

    nc.compile()
    return nc


_NC_CACHE = {}


def _get_nc(cap: int):
    if cap not in _NC_CACHE:
        _NC_CACHE[cap] = _build(cap)
    return _NC_CACHE[cap]


def plan(x, router_w, router_b):
    """Host-side routing: token lists per expert, gate values, capacity."""
    xt = x.reshape(-1, H)
    logits = xt @ router_w + router_b
    idx = logits.argmax(-1)
    mx = logits.max(-1)
    gate = 1.0 / np.exp(logits - mx[:, None]).sum(-1)
    toks = [np.where(idx == e)[0] for e in range(E)]
    cap = max(P, -(-max(len(t) for t in toks) // 64) * 64)
    return toks, gate.astype(np.float32), cap


def make_in_maps(x, expert_w, toks, gate, cap):
    import ml_dtypes
    bf = ml_dtypes.bfloat16
    xt = x.reshape(-1, H)
    maps = []
    for e in range(E):
        te = toks[e]
        xs = np.zeros((KC, P, cap), dtype=bf)
        xs.reshape(H, cap)[:, :len(te)] = (xt[te] * gate[te, None]).T.astype(bf)
        w = expert_w[e].astype(bf).reshape(KC, P, H)
        w_arr = w.transpose(1, 0, 2).reshape(P, KC * H)
        x_arr = xs.transpose(1, 0, 2).reshape(P, KC * cap)
        maps.append({
            "wx0": np.ascontiguousarray(
                np.concatenate([w_arr[:, :H], x_arr[:, :512]], axis=1)),
            "w": np.ascontiguousarray(w_arr[:, H:]),
            "xt": np.ascontiguousarray(x_arr[:, 512:]),
        })
    return maps


def assemble(results, toks, gate, expert_b, shape):
    T = shape[0] * shape[1]
    y = np.empty((T, H), dtype=np.float32)
    for e in range(E):
        te = toks[e]
        y[te] = results[e]["y"][:len(te)]
        if expert_b is not None:
            y[te] += gate[te, None] * expert_b[e][None, :]
    return y.reshape(shape)


def kernel(x, router_w, router_b, expert_w, expert_b):
    from concourse.bass_utils import run_bass_kernel_spmd

    x = np.ascontiguousarray(np.asarray(x, dtype=np.float32))
    router_w = np.ascontiguousarray(np.asarray(router_w, dtype=np.float32))
    router_b = np.ascontiguousarray(np.asarray(router_b, dtype=np.float32))
    expert_w = np.ascontiguousarray(np.asarray(expert_w, dtype=np.float32))
    expert_b = np.ascontiguousarray(np.asarray(expert_b, dtype=np.float32))

    B, S, Hx = x.shape
    assert Hx == H and B * S % NCORES == 0, (x.shape,)

    toks, gate, cap = plan(x, router_w, router_b)
    nc = _get_nc(cap)
    in_maps = make_in_maps(x, expert_w, toks, gate, cap)
    res = run_bass_kernel_spmd(nc, in_maps, list(range(NCORES)))
    eb = expert_b if np.any(expert_b != 0) else None
    return assemble(res.results, toks, gate, eb, (B, S, H))


# revision 27
# speedup vs baseline: 1.1550x; 1.0042x over previous
"""Trainium2 Bass kernel for nn_ExpertFFN (top-1 MoE, B=4 S=2048 H=1024 E=8).

Strategy: EXPERT parallelism.  Core c owns expert c's weights only (bf16,
2 MB instead of 32 MB replicated), and the host does all routing:

  host:   logits = x @ router_w + router_b (fp32, same as the reference),
          idx = argmax, gate = softmax max = 1/sum(exp(l - max)).
          Tokens for expert e are gathered, pre-scaled by gate (y =
          gate*(x@W+b) = (gate*x)@W + gate*b), transposed to feature-major,
          cast to bf16, zero-padded to a shared capacity, and laid out
          k-chunk-major ([128, KC*n]) so the device loads each stream with
          4 large contiguous DMAs (sizes 1/1/2/4 chunks: fine-grained at
          the front for early compute start, big at the back for
          bandwidth; 8 HWDGE DMAs in flight total avoids sem-lane reuse
          stalls).
  device: pure single-expert GEMM  y[cap, H] = xT.T @ W  in bf16 with fp32
          PSUM accumulation.  Tokens ride the stationary operand (lhsT =
          xT column tile), weight columns stream as rhs (2 x 512-col PSUM
          banks per token tile).  The first 4 token tiles run (k-chunk)-
          major so each arriving chunk feeds ~2us of PE work; remaining
          tiles run t-major so evacuation (vector/scalar halves, y halves
          DMA'd on the gpsimd + sync queues) hides behind the next tile's
          matmuls and the kernel tail is one half-tile evacuation.
  host:   scatter y rows back by token index, add gate*expert_b, unshard.

Pure-bf16 precision measured at rel err 2.3e-3 vs the fp32 reference
(tolerance 2e-2).
"""

import sys

for _p in ("/opt/trn_rl_repo",):
    if _p not in sys.path:
        sys.path.insert(0, _p)

import numpy as np

P = 128
H = 1024
E = 8
NCORES = 8
KC = H // P          # contraction chunks
GRP = 4              # token tiles in the k-major (DMA-overlap) group
SPLITS = (1, 1, 1, 1, 4)  # chunks per input DMA (first = combined w0+x0)


def _build(cap: int):
    import concourse.mybir as mybir
    import concourse.tile as tile
    from concourse import bacc

    f32 = mybir.dt.float32
    bf16 = mybir.dt.bfloat16

    ntt = (cap + P - 1) // P

    nc = bacc.Bacc("TRN2", target_bir_lowering=False, debug=False,
                   num_devices=NCORES)

    # host pre-arranged chunk-major layouts: [p, k*N + j] = src[k*128+p, j].
    # The critical first transfer is one DMA carrying w chunk 0 plus the
    # first 512 token columns of x chunk 0 (all that the k-major group
    # needs); x0's remaining columns lead the second stream.
    X0B = cap - 512
    wx0_d = nc.dram_tensor("wx0", [P, H + 512], bf16, kind="ExternalInput")
    w_d = nc.dram_tensor("w", [P, (KC - 1) * H], bf16, kind="ExternalInput")
    xt_d = nc.dram_tensor("xt", [P, X0B + (KC - 1) * cap], bf16,
                          kind="ExternalInput")
    y_d = nc.dram_tensor("y", [cap, H], f32, kind="ExternalOutput")

    with tile.TileContext(nc) as tc:
        with (
            tc.tile_pool(name="consts", bufs=1) as cpool,
            tc.tile_pool(name="wp", bufs=len(SPLITS)) as wpool,
            tc.tile_pool(name="xp", bufs=len(SPLITS)) as xpool,
            tc.tile_pool(name="yp", bufs=ntt + 1) as ypool,
            tc.tile_pool(name="ps", bufs=GRP, space="PSUM") as pspool,
        ):
            # HAM warmup: dummy matmul activity from t=0 so the PE clock
            # gate opens while the input DMAs land; memset on vector (its
            # IRAM loads early) so the warmup starts ~5us, sized to span
            # until the first weight chunk arrives (~11.5us)
            warm = cpool.tile([P, 128], bf16)
            nc.vector.memset(warm[:], 0.0)
            pw = pspool.tile([P, 128], f32, tag="ps", space="PSUM")
            NWARM = 40
            for i in range(NWARM):
                nc.tensor.matmul(out=pw[:], lhsT=warm[:], rhs=warm[:],
                                 start=(i == 0), stop=(i == NWARM - 1))

            # graduated input DMAs on the two HWDGE rings: the combined
            # (w0, x0-front) block first on sync, then chunks 1..7 in
            # splits (1, 1, 1, 4) per stream; the first scalar DMA also
            # carries x0's remaining token columns
            wx0 = wpool.tile([P, H + 512], bf16, tag="wx0", name="wx0")
            nc.sync.dma_start(out=wx0[:], in_=wx0_d[:, :])
            wq, xq, qof = [wx0], [None], [0]
            off = 0
            for si, s in enumerate(SPLITS[1:]):
                wt = wpool.tile([P, s * H], bf16, tag=f"w{off}",
                                name=f"w{off}")
                nc.sync.dma_start(out=wt[:],
                                  in_=w_d[:, off * H:(off + s) * H])
                xb = X0B if si == 0 else 0
                xt = xpool.tile([P, xb + s * cap], bf16, tag=f"x{off}",
                                name=f"x{off}")
                nc.scalar.dma_start(
                    out=xt[:],
                    in_=xt_d[:, off * cap + (X0B - xb):
                             X0B + (off + s) * cap])
                if si == 0:
                    x0b = xt
                for _ in range(s):
                    wq.append(wt)
                    xq.append(xt)
                    qof.append(off)
                off += s

            def wk(k):
                if k == 0:
                    return wx0[:, 0:H]
                return wq[k][:, (k - 1 - qof[k]) * H:(k - qof[k]) * H]

            def xk(k, tcols):
                # chunk k's token columns [tcols.start, tcols.stop)
                if k == 0:
                    if tcols.stop <= 512:
                        return wx0[:, H + tcols.start:H + tcols.stop]
                    return x0b[:, tcols.start - 512:tcols.stop - 512]
                base = (X0B if qof[k] == 0 else 0) + (k - 1 - qof[k]) * cap
                return xq[k][:, base + tcols.start:base + tcols.stop]

            ps = {}

            def evac(t, last=False):
                tw = min(P, cap - t * P)
                yb = ypool.tile([P, H], f32, tag="y", name=f"y{t}")
                nc.vector.tensor_copy(out=yb[0:tw, 0:512],
                                      in_=ps[t][0:tw, 0:512])
                nc.scalar.copy(out=yb[0:tw, 512:H], in_=ps[t][0:tw, 512:H])
                if last:
                    # half-column writes on the two low-latency HWDGE
                    # queues (one DMA issue each) so the tail is short
                    nc.sync.dma_start(out=y_d[t * P:t * P + tw, 0:512],
                                      in_=yb[0:tw, 0:512])
                    nc.scalar.dma_start(out=y_d[t * P:t * P + tw, 512:H],
                                        in_=yb[0:tw, 512:H])
                else:
                    nc.scalar.dma_start(out=y_d[t * P:t * P + tw, 0:512],
                                        in_=yb[0:tw, 0:512])
                    nc.sync.dma_start(out=y_d[t * P:t * P + tw, 512:H],
                                      in_=yb[0:tw, 512:H])

            # group A: k-major over the first GRP token tiles so each
            # arriving chunk feeds 8 matmuls
            ga = range(0, min(GRP, ntt))
            for t in ga:
                ps[t] = pspool.tile([P, H], f32, tag="ps", space="PSUM",
                                    name=f"ps{t}")
            for k in range(KC):
                for t in ga:
                    tw = min(P, cap - t * P)
                    for n in range(2):
                        nc.tensor.matmul(
                            out=ps[t][0:tw, n * 512:(n + 1) * 512],
                            lhsT=xk(k, slice(t * P, t * P + tw)),
                            rhs=wk(k)[:, n * 512:(n + 1) * 512],
                            start=(k == 0), stop=(k == KC - 1))
            for t in ga:
                evac(t)

            # remaining tiles: t-major; evacuation hides behind the next
            # tile's matmuls
            for t in range(GRP, ntt):
                tw = min(P, cap - t * P)
                ps[t] = pspool.tile([P, H], f32, tag="ps", space="PSUM",
                                    name=f"ps{t}")
                for k in range(KC):
                    for n in range(2):
                        nc.tensor.matmul(
                            out=ps[t][0:tw, n * 512:(n + 1) * 512],
                            lhsT=xk(k, slice(t * P, t * P + tw)),
                            rhs=wk(k)[:, n * 512:(n + 1) * 512],
                            start=(k == 0), stop=(k == KC - 1))
                evac(t, last=(t == ntt - 1))

    nc.compile()
    return nc


_NC_CACHE = {}


def _get_nc(cap: int):
    if cap not in _NC_CACHE:
        _NC_CACHE[cap] = _build(cap)
    return _NC_CACHE[cap]


def plan(x, router_w, router_b):
    """Host-side routing: token lists per expert, gate values, capacity."""
    xt = x.reshape(-1, H)
    logits = xt @ router_w + router_b
    idx = logits.argmax(-1)
    mx = logits.max(-1)
    gate = 1.0 / np.exp(logits - mx[:, None]).sum(-1)
    toks = [np.where(idx == e)[0] for e in range(E)]
    cap = max(P, -(-max(len(t) for t in toks) // 64) * 64)
    return toks, gate.astype(np.float32), cap


def make_in_maps(x, expert_w, toks, gate, cap):
    import ml_dtypes
    bf = ml_dtypes.bfloat16
    xt = x.reshape(-1, H)
    maps = []
    for e in range(E):
        te = toks[e]
        xs = np.zeros((KC, P, cap), dtype=bf)
        xs.reshape(H, cap)[:, :len(te)] = (xt[te] * gate[te, None]).T.astype(bf)
        w = expert_w[e].astype(bf).reshape(KC, P, H)
        w_arr = w.transpose(1, 0, 2).reshape(P, KC * H)
        x_arr = xs.transpose(1, 0, 2).reshape(P, KC * cap)
        maps.append({
            "wx0": np.ascontiguousarray(
                np.concatenate([w_arr[:, :H], x_arr[:, :512]], axis=1)),
            "w": np.ascontiguousarray(w_arr[:, H:]),
            "xt": np.ascontiguousarray(x_arr[:, 512:]),
        })
    return maps


def assemble(results, toks, gate, expert_b, shape):
    T = shape[0] * shape[1]
    y = np.empty((T, H), dtype=np.float32)
    for e in range(E):
        te = toks[e]
        y[te] = results[e]["y"][:len(te)]
        if expert_b is not None:
            y[te] += gate[te, None] * expert_b[e][None, :]
    return y.reshape(shape)


def kernel(x, router_w, router_b, expert_w, expert_b):
    from concourse.bass_utils import run_bass_kernel_spmd

    x = np.ascontiguousarray(np.asarray(x, dtype=np.float32))
    router_w = np.ascontiguousarray(np.asarray(router_w, dtype=np.float32))
    router_b = np.ascontiguousarray(np.asarray(router_b, dtype=np.float32))
    expert_w = np.ascontiguousarray(np.asarray(expert_w, dtype=np.float32))
    expert_b = np.ascontiguousarray(np.asarray(expert_b, dtype=np.float32))

    B, S, Hx = x.shape
    assert Hx == H and B * S % NCORES == 0, (x.shape,)

    toks, gate, cap = plan(x, router_w, router_b)
    nc = _get_nc(cap)
    in_maps = make_in_maps(x, expert_w, toks, gate, cap)
    res = run_bass_kernel_spmd(nc, in_maps, list(range(NCORES)))
    eb = expert_b if np.any(expert_b != 0) else None
    return assemble(res.results, toks, gate, eb, (B, S, H))


# revision 30
# speedup vs baseline: 1.2023x; 1.0410x over previous
"""Trainium2 Bass kernel for nn_ExpertFFN (top-1 MoE, B=4 S=2048 H=1024 E=8).

Strategy: EXPERT parallelism.  Core c owns expert c's weights only (bf16,
2 MB instead of 32 MB replicated), and the host does all routing:

  host:   logits = x @ router_w + router_b (fp32, same as the reference),
          idx = argmax, gate = softmax max = 1/sum(exp(l - max)).
          Tokens for expert e are gathered, pre-scaled by gate (y =
          gate*(x@W+b) = (gate*x)@W + gate*b), transposed to feature-major,
          cast to bf16, zero-padded to a shared capacity, and laid out
          k-chunk-major ([128, KC*n]) so the device loads each stream with
          4 large contiguous DMAs (sizes 1/1/2/4 chunks: fine-grained at
          the front for early compute start, big at the back for
          bandwidth; 8 HWDGE DMAs in flight total avoids sem-lane reuse
          stalls).
  device: pure single-expert GEMM  y[cap, H] = xT.T @ W  in bf16 with fp32
          PSUM accumulation.  Tokens ride the stationary operand (lhsT =
          xT column tile), weight columns stream as rhs (2 x 512-col PSUM
          banks per token tile).  The first 4 token tiles run (k-chunk)-
          major so each arriving chunk feeds ~2us of PE work; remaining
          tiles run t-major so evacuation (vector/scalar halves, y halves
          DMA'd on the gpsimd + sync queues) hides behind the next tile's
          matmuls and the kernel tail is one half-tile evacuation.
  host:   scatter y rows back by token index, add gate*expert_b, unshard.

Pure-bf16 precision measured at rel err 2.3e-3 vs the fp32 reference
(tolerance 2e-2).
"""

import sys

for _p in ("/opt/trn_rl_repo",):
    if _p not in sys.path:
        sys.path.insert(0, _p)

import numpy as np

P = 128
H = 1024
E = 8
NCORES = 8
KC = H // P          # contraction chunks
GRP = 4              # token tiles in the k-major (DMA-overlap) group
SPLITS = (1, 1, 1, 1, 4)  # chunks per input DMA (first = combined w0+x0)


def _build(cap: int):
    import concourse.mybir as mybir
    import concourse.tile as tile
    from concourse import bacc

    f32 = mybir.dt.float32
    bf16 = mybir.dt.bfloat16

    ntt = (cap + P - 1) // P

    nc = bacc.Bacc("TRN2", target_bir_lowering=False, debug=False,
                   num_devices=NCORES)

    # host pre-arranged chunk-major byte streams, interleaved across the
    # two HWDGE rings so chunk pairs (w_k, x_k) land alternately with
    # slack over the PE's consumption pace:
    #   ring A (sync):   [w0|x0(:512)] [w2|x2] [w4|x4] [w6|x6]
    #   ring B (scalar): [w1|x0(512:)|x1] [w3|x3] [w5|x5] [w7|x7]
    X0B = cap - 512
    sa_d = nc.dram_tensor("sa", [P, H + 512 + 3 * (H + cap)], bf16,
                          kind="ExternalInput")
    sb_d = nc.dram_tensor("sb", [P, H + X0B + cap + 3 * (H + cap)], bf16,
                          kind="ExternalInput")
    y_d = nc.dram_tensor("y", [cap, H], f32, kind="ExternalOutput")

    with tile.TileContext(nc) as tc:
        with (
            tc.tile_pool(name="consts", bufs=1) as cpool,
            tc.tile_pool(name="wp", bufs=len(SPLITS)) as wpool,
            tc.tile_pool(name="xp", bufs=len(SPLITS)) as xpool,
            tc.tile_pool(name="yp", bufs=ntt + 1) as ypool,
            tc.tile_pool(name="ps", bufs=GRP, space="PSUM") as pspool,
        ):
            # HAM warmup: dummy matmul activity from t=0 so the PE clock
            # gate opens while the input DMAs land; memset on vector (its
            # IRAM loads early) so the warmup starts ~5us, sized to span
            # until the first weight chunk arrives (~11.5us)
            warm = cpool.tile([P, 128], bf16)
            nc.vector.memset(warm[:], 0.0)
            pw = pspool.tile([P, 128], f32, tag="ps", space="PSUM")
            NWARM = 40
            for i in range(NWARM):
                nc.tensor.matmul(out=pw[:], lhsT=warm[:], rhs=warm[:],
                                 start=(i == 0), stop=(i == NWARM - 1))

            # four block DMAs per ring, in chunk order
            at, bt = [], []
            aoff = boff = 0
            for bi in range(4):
                aw = H + 512 if bi == 0 else H + cap
                bw = H + X0B + cap if bi == 0 else H + cap
                ta = wpool.tile([P, aw], bf16, tag=f"a{bi}", name=f"a{bi}")
                nc.sync.dma_start(out=ta[:], in_=sa_d[:, aoff:aoff + aw])
                tb = xpool.tile([P, bw], bf16, tag=f"b{bi}", name=f"b{bi}")
                nc.scalar.dma_start(out=tb[:], in_=sb_d[:, boff:boff + bw])
                at.append(ta)
                bt.append(tb)
                aoff += aw
                boff += bw

            def wk(k):
                tile = at[k // 2] if k % 2 == 0 else bt[k // 2]
                return tile[:, 0:H]

            def xk(k, tc):
                # chunk k's token columns [tc.start, tc.stop)
                if k == 0:
                    if tc.stop <= 512:
                        return at[0][:, H + tc.start:H + tc.stop]
                    return bt[0][:, H + tc.start - 512:H + tc.stop - 512]
                if k == 1:
                    base = H + X0B
                    return bt[0][:, base + tc.start:base + tc.stop]
                tile = at[k // 2] if k % 2 == 0 else bt[k // 2]
                return tile[:, H + tc.start:H + tc.stop]

            ps = {}

            def evac(t, last=False):
                tw = min(P, cap - t * P)
                yb = ypool.tile([P, H], f32, tag="y", name=f"y{t}")
                nc.vector.tensor_copy(out=yb[0:tw, 0:512],
                                      in_=ps[t][0:tw, 0:512])
                nc.scalar.copy(out=yb[0:tw, 512:H], in_=ps[t][0:tw, 512:H])
                if last:
                    # half-column writes on the two low-latency HWDGE
                    # queues (one DMA issue each) so the tail is short
                    nc.sync.dma_start(out=y_d[t * P:t * P + tw, 0:512],
                                      in_=yb[0:tw, 0:512])
                    nc.scalar.dma_start(out=y_d[t * P:t * P + tw, 512:H],
                                        in_=yb[0:tw, 512:H])
                else:
                    nc.scalar.dma_start(out=y_d[t * P:t * P + tw, 0:512],
                                        in_=yb[0:tw, 0:512])
                    nc.sync.dma_start(out=y_d[t * P:t * P + tw, 512:H],
                                      in_=yb[0:tw, 512:H])

            # group A: k-major over the first GRP token tiles so each
            # arriving chunk feeds 8 matmuls
            ga = range(0, min(GRP, ntt))
            for t in ga:
                ps[t] = pspool.tile([P, H], f32, tag="ps", space="PSUM",
                                    name=f"ps{t}")
            for k in range(KC):
                for t in ga:
                    tw = min(P, cap - t * P)
                    for n in range(2):
                        nc.tensor.matmul(
                            out=ps[t][0:tw, n * 512:(n + 1) * 512],
                            lhsT=xk(k, slice(t * P, t * P + tw)),
                            rhs=wk(k)[:, n * 512:(n + 1) * 512],
                            start=(k == 0), stop=(k == KC - 1))
            for t in ga:
                evac(t)

            # remaining tiles: t-major; evacuation hides behind the next
            # tile's matmuls
            for t in range(GRP, ntt):
                tw = min(P, cap - t * P)
                ps[t] = pspool.tile([P, H], f32, tag="ps", space="PSUM",
                                    name=f"ps{t}")
                for k in range(KC):
                    for n in range(2):
                        nc.tensor.matmul(
                            out=ps[t][0:tw, n * 512:(n + 1) * 512],
                            lhsT=xk(k, slice(t * P, t * P + tw)),
                            rhs=wk(k)[:, n * 512:(n + 1) * 512],
                            start=(k == 0), stop=(k == KC - 1))
                evac(t, last=(t == ntt - 1))

    nc.compile()
    return nc


_NC_CACHE = {}


def _get_nc(cap: int):
    if cap not in _NC_CACHE:
        _NC_CACHE[cap] = _build(cap)
    return _NC_CACHE[cap]


def plan(x, router_w, router_b):
    """Host-side routing: token lists per expert, gate values, capacity."""
    xt = x.reshape(-1, H)
    logits = xt @ router_w + router_b
    idx = logits.argmax(-1)
    mx = logits.max(-1)
    gate = 1.0 / np.exp(logits - mx[:, None]).sum(-1)
    toks = [np.where(idx == e)[0] for e in range(E)]
    cap = max(P, -(-max(len(t) for t in toks) // 64) * 64)
    return toks, gate.astype(np.float32), cap


def make_in_maps(x, expert_w, toks, gate, cap):
    import ml_dtypes
    bf = ml_dtypes.bfloat16
    xt = x.reshape(-1, H)
    maps = []
    for e in range(E):
        te = toks[e]
        xs = np.zeros((KC, P, cap), dtype=bf)
        xs.reshape(H, cap)[:, :len(te)] = (xt[te] * gate[te, None]).T.astype(bf)
        w = expert_w[e].astype(bf).reshape(KC, P, H)
        w_arr = w.transpose(1, 0, 2).reshape(P, KC * H)
        x_arr = xs.transpose(1, 0, 2).reshape(P, KC * cap)

        def wc(k):
            return w_arr[:, k * H:(k + 1) * H]

        def xc(k):
            return x_arr[:, k * cap:(k + 1) * cap]

        sa = np.concatenate(
            [wc(0), x_arr[:, 0:512]] +
            sum([[wc(k), xc(k)] for k in (2, 4, 6)], []), axis=1)
        sb = np.concatenate(
            [wc(1), x_arr[:, 512:cap], xc(1)] +
            sum([[wc(k), xc(k)] for k in (3, 5, 7)], []), axis=1)
        maps.append({
            "sa": np.ascontiguousarray(sa),
            "sb": np.ascontiguousarray(sb),
        })
    return maps


def assemble(results, toks, gate, expert_b, shape):
    T = shape[0] * shape[1]
    y = np.empty((T, H), dtype=np.float32)
    for e in range(E):
        te = toks[e]
        y[te] = results[e]["y"][:len(te)]
        if expert_b is not None:
            y[te] += gate[te, None] * expert_b[e][None, :]
    return y.reshape(shape)


def kernel(x, router_w, router_b, expert_w, expert_b):
    from concourse.bass_utils import run_bass_kernel_spmd

    x = np.ascontiguousarray(np.asarray(x, dtype=np.float32))
    router_w = np.ascontiguousarray(np.asarray(router_w, dtype=np.float32))
    router_b = np.ascontiguousarray(np.asarray(router_b, dtype=np.float32))
    expert_w = np.ascontiguousarray(np.asarray(expert_w, dtype=np.float32))
    expert_b = np.ascontiguousarray(np.asarray(expert_b, dtype=np.float32))

    B, S, Hx = x.shape
    assert Hx == H and B * S % NCORES == 0, (x.shape,)

    toks, gate, cap = plan(x, router_w, router_b)
    nc = _get_nc(cap)
    in_maps = make_in_maps(x, expert_w, toks, gate, cap)
    res = run_bass_kernel_spmd(nc, in_maps, list(range(NCORES)))
    eb = expert_b if np.any(expert_b != 0) else None
    return assemble(res.results, toks, gate, eb, (B, S, H))


# revision 31
# speedup vs baseline: 1.2136x; 1.0094x over previous
"""Trainium2 Bass kernel for nn_ExpertFFN (top-1 MoE, B=4 S=2048 H=1024 E=8).

Strategy: EXPERT parallelism.  Core c owns expert c's weights only (bf16,
2 MB instead of 32 MB replicated), and the host does all routing:

  host:   logits = x @ router_w + router_b (fp32, same as the reference),
          idx = argmax, gate = softmax max = 1/sum(exp(l - max)).
          Tokens for expert e are gathered, pre-scaled by gate (y =
          gate*(x@W+b) = (gate*x)@W + gate*b), transposed to feature-major,
          cast to bf16, zero-padded to a shared capacity, and laid out
          k-chunk-major ([128, KC*n]) so the device loads each stream with
          4 large contiguous DMAs (sizes 1/1/2/4 chunks: fine-grained at
          the front for early compute start, big at the back for
          bandwidth; 8 HWDGE DMAs in flight total avoids sem-lane reuse
          stalls).
  device: pure single-expert GEMM  y[cap, H] = xT.T @ W  in bf16 with fp32
          PSUM accumulation.  Tokens ride the stationary operand (lhsT =
          xT column tile), weight columns stream as rhs (2 x 512-col PSUM
          banks per token tile).  The first 4 token tiles run (k-chunk)-
          major so each arriving chunk feeds ~2us of PE work; remaining
          tiles run t-major so evacuation (vector/scalar halves, y halves
          DMA'd on the gpsimd + sync queues) hides behind the next tile's
          matmuls and the kernel tail is one half-tile evacuation.
  host:   scatter y rows back by token index, add gate*expert_b, unshard.

Pure-bf16 precision measured at rel err 2.3e-3 vs the fp32 reference
(tolerance 2e-2).
"""

import sys

for _p in ("/opt/trn_rl_repo",):
    if _p not in sys.path:
        sys.path.insert(0, _p)

import numpy as np

P = 128
H = 1024
E = 8
NCORES = 8
KC = H // P          # contraction chunks
GRP = 4              # token tiles in the k-major (DMA-overlap) group
SPLITS = (1, 1, 1, 1, 4)  # chunks per input DMA (first = combined w0+x0)


def _build(cap: int):
    import concourse.mybir as mybir
    import concourse.tile as tile
    from concourse import bacc

    f32 = mybir.dt.float32
    bf16 = mybir.dt.bfloat16

    ntt = (cap + P - 1) // P

    nc = bacc.Bacc("TRN2", target_bir_lowering=False, debug=False,
                   num_devices=NCORES)

    # host pre-arranged chunk-major byte streams, interleaved across the
    # two HWDGE rings so chunk pairs (w_k, x_k) land alternately with
    # slack over the PE's consumption pace:
    #   ring A (sync):   [w0|x0(:512)] [w2|x2] [w4|x4] [w6|x6]
    #   ring B (scalar): [w1|x0(512:)|x1] [w3|x3] [w5|x5] [w7|x7]
    X0B = cap - 512
    sa_d = nc.dram_tensor("sa", [P, H + 512 + 3 * (H + cap)], bf16,
                          kind="ExternalInput")
    sb_d = nc.dram_tensor("sb", [P, H + X0B + cap + 3 * (H + cap)], bf16,
                          kind="ExternalInput")
    y_d = nc.dram_tensor("y", [cap, H], f32, kind="ExternalOutput")

    with tile.TileContext(nc) as tc:
        with (
            tc.tile_pool(name="consts", bufs=1) as cpool,
            tc.tile_pool(name="wp", bufs=len(SPLITS)) as wpool,
            tc.tile_pool(name="xp", bufs=len(SPLITS)) as xpool,
            tc.tile_pool(name="yp", bufs=ntt + 1) as ypool,
            tc.tile_pool(name="ps", bufs=GRP, space="PSUM") as pspool,
        ):
            # HAM warmup: dummy matmul activity from t=0 so the PE clock
            # gate opens while the input DMAs land; memset on vector (its
            # IRAM loads early) so the warmup starts ~5us, sized to span
            # until the first weight chunk arrives (~11.5us)
            warm = cpool.tile([P, 128], bf16)
            nc.vector.memset(warm[:], 0.0)
            pw = pspool.tile([P, 128], f32, tag="ps", space="PSUM")
            NWARM = 40
            for i in range(NWARM):
                nc.tensor.matmul(out=pw[:], lhsT=warm[:], rhs=warm[:],
                                 start=(i == 0), stop=(i == NWARM - 1))

            # four block DMAs per ring, in chunk order
            at, bt = [], []
            aoff = boff = 0
            for bi in range(4):
                aw = H + 512 if bi == 0 else H + cap
                bw = H + X0B + cap if bi == 0 else H + cap
                ta = wpool.tile([P, aw], bf16, tag=f"a{bi}", name=f"a{bi}")
                nc.sync.dma_start(out=ta[:], in_=sa_d[:, aoff:aoff + aw])
                tb = xpool.tile([P, bw], bf16, tag=f"b{bi}", name=f"b{bi}")
                nc.scalar.dma_start(out=tb[:], in_=sb_d[:, boff:boff + bw])
                at.append(ta)
                bt.append(tb)
                aoff += aw
                boff += bw

            def wk(k):
                tile = at[k // 2] if k % 2 == 0 else bt[k // 2]
                return tile[:, 0:H]

            def xk(k, tc):
                # chunk k's token columns [tc.start, tc.stop)
                if k == 0:
                    if tc.stop <= 512:
                        return at[0][:, H + tc.start:H + tc.stop]
                    return bt[0][:, H + tc.start - 512:H + tc.stop - 512]
                if k == 1:
                    base = H + X0B
                    return bt[0][:, base + tc.start:base + tc.stop]
                tile = at[k // 2] if k % 2 == 0 else bt[k // 2]
                return tile[:, H + tc.start:H + tc.stop]

            ps = {}

            def evac(t, last=False):
                tw = min(P, cap - t * P)
                yb = ypool.tile([P, H], f32, tag="y", name=f"y{t}")
                nc.vector.tensor_copy(out=yb[0:tw, 0:512],
                                      in_=ps[t][0:tw, 0:512])
                nc.scalar.copy(out=yb[0:tw, 512:H], in_=ps[t][0:tw, 512:H])
                if last:
                    # half-column writes on the two low-latency HWDGE
                    # queues (one DMA issue each) so the tail is short
                    nc.sync.dma_start(out=y_d[t * P:t * P + tw, 0:512],
                                      in_=yb[0:tw, 0:512])
                    nc.scalar.dma_start(out=y_d[t * P:t * P + tw, 512:H],
                                        in_=yb[0:tw, 512:H])
                else:
                    nc.scalar.dma_start(out=y_d[t * P:t * P + tw, 0:512],
                                        in_=yb[0:tw, 0:512])
                    nc.sync.dma_start(out=y_d[t * P:t * P + tw, 512:H],
                                      in_=yb[0:tw, 512:H])

            # group A: k-major over the first GRP token tiles so each
            # arriving chunk feeds 8 matmuls
            ga = range(0, min(GRP, ntt))
            for t in ga:
                ps[t] = pspool.tile([P, H], f32, tag="ps", space="PSUM",
                                    name=f"ps{t}")
            for k in range(KC):
                for t in ga:
                    tw = min(P, cap - t * P)
                    for n in range(2):
                        nc.tensor.matmul(
                            out=ps[t][0:tw, n * 512:(n + 1) * 512],
                            lhsT=xk(k, slice(t * P, t * P + tw)),
                            rhs=wk(k)[:, n * 512:(n + 1) * 512],
                            start=(k == 0), stop=(k == KC - 1))
            for t in ga:
                evac(t)

            # remaining tiles: t-major; evacuation hides behind the next
            # tile's matmuls.  The final tile runs its two 512-col PSUM
            # banks as separate k-sweeps so the n=0 half evacuates (on
            # the busier scalar ring) while n=1 is still accumulating,
            # leaving only the n=1 half (on the lighter sync ring) in
            # the kernel tail.
            for t in range(GRP, ntt):
                tw = min(P, cap - t * P)
                ps[t] = pspool.tile([P, H], f32, tag="ps", space="PSUM",
                                    name=f"ps{t}")
                last = t == ntt - 1
                if last:
                    yb = ypool.tile([P, H], f32, tag="y", name=f"y{t}")
                    for n in range(2):
                        for k in range(KC):
                            nc.tensor.matmul(
                                out=ps[t][0:tw, n * 512:(n + 1) * 512],
                                lhsT=xk(k, slice(t * P, t * P + tw)),
                                rhs=wk(k)[:, n * 512:(n + 1) * 512],
                                start=(k == 0), stop=(k == KC - 1))
                        cs = slice(n * 512, (n + 1) * 512)
                        if n == 0:
                            nc.scalar.copy(out=yb[0:tw, cs],
                                           in_=ps[t][0:tw, cs])
                            nc.scalar.dma_start(
                                out=y_d[t * P:t * P + tw, cs],
                                in_=yb[0:tw, cs])
                        else:
                            nc.vector.tensor_copy(out=yb[0:tw, cs],
                                                  in_=ps[t][0:tw, cs])
                            nc.sync.dma_start(
                                out=y_d[t * P:t * P + tw, cs],
                                in_=yb[0:tw, cs])
                else:
                    for k in range(KC):
                        for n in range(2):
                            nc.tensor.matmul(
                                out=ps[t][0:tw, n * 512:(n + 1) * 512],
                                lhsT=xk(k, slice(t * P, t * P + tw)),
                                rhs=wk(k)[:, n * 512:(n + 1) * 512],
                                start=(k == 0), stop=(k == KC - 1))
                    evac(t)

    nc.compile()
    return nc


_NC_CACHE = {}


def _get_nc(cap: int):
    if cap not in _NC_CACHE:
        _NC_CACHE[cap] = _build(cap)
    return _NC_CACHE[cap]


def plan(x, router_w, router_b):
    """Host-side routing: token lists per expert, gate values, capacity."""
    xt = x.reshape(-1, H)
    logits = xt @ router_w + router_b
    idx = logits.argmax(-1)
    mx = logits.max(-1)
    gate = 1.0 / np.exp(logits - mx[:, None]).sum(-1)
    toks = [np.where(idx == e)[0] for e in range(E)]
    cap = max(P, -(-max(len(t) for t in toks) // 64) * 64)
    return toks, gate.astype(np.float32), cap


def make_in_maps(x, expert_w, toks, gate, cap):
    import ml_dtypes
    bf = ml_dtypes.bfloat16
    xt = x.reshape(-1, H)
    maps = []
    for e in range(E):
        te = toks[e]
        xs = np.zeros((KC, P, cap), dtype=bf)
        xs.reshape(H, cap)[:, :len(te)] = (xt[te] * gate[te, None]).T.astype(bf)
        w = expert_w[e].astype(bf).reshape(KC, P, H)
        w_arr = w.transpose(1, 0, 2).reshape(P, KC * H)
        x_arr = xs.transpose(1, 0, 2).reshape(P, KC * cap)

        def wc(k):
            return w_arr[:, k * H:(k + 1) * H]

        def xc(k):
            return x_arr[:, k * cap:(k + 1) * cap]

        sa = np.concatenate(
            [wc(0), x_arr[:, 0:512]] +
            sum([[wc(k), xc(k)] for k in (2, 4, 6)], []), axis=1)
        sb = np.concatenate(
            [wc(1), x_arr[:, 512:cap], xc(1)] +
            sum([[wc(k), xc(k)] for k in (3, 5, 7)], []), axis=1)
        maps.append({
            "sa": np.ascontiguousarray(sa),
            "sb": np.ascontiguousarray(sb),
        })
    return maps


def assemble(results, toks, gate, expert_b, shape):
    T = shape[0] * shape[1]
    y = np.empty((T, H), dtype=np.float32)
    for e in range(E):
        te = toks[e]
        y[te] = results[e]["y"][:len(te)]
        if expert_b is not None:
            y[te] += gate[te, None] * expert_b[e][None, :]
    return y.reshape(shape)


def kernel(x, router_w, router_b, expert_w, expert_b):
    from concourse.bass_utils import run_bass_kernel_spmd

    x = np.ascontiguousarray(np.asarray(x, dtype=np.float32))
    router_w = np.ascontiguousarray(np.asarray(router_w, dtype=np.float32))
    router_b = np.ascontiguousarray(np.asarray(router_b, dtype=np.float32))
    expert_w = np.ascontiguousarray(np.asarray(expert_w, dtype=np.float32))
    expert_b = np.ascontiguousarray(np.asarray(expert_b, dtype=np.float32))

    B, S, Hx = x.shape
    assert Hx == H and B * S % NCORES == 0, (x.shape,)

    toks, gate, cap = plan(x, router_w, router_b)
    nc = _get_nc(cap)
    in_maps = make_in_maps(x, expert_w, toks, gate, cap)
    res = run_bass_kernel_spmd(nc, in_maps, list(range(NCORES)))
    eb = expert_b if np.any(expert_b != 0) else None
    return assemble(res.results, toks, gate, eb, (B, S, H))
